# revision 12
# baseline (speedup 1.0000x reference)
"""ContextAttention TRN2 kernel: out, attn = softmax(mask(inp@Wk^T @ inp^T / sqrt(D))) ...

Full-input contract: kernel(**inputs) takes the complete tensors, shards
across 8 NeuronCores (2 cores per batch element, each handling 2048 query
rows), runs one SPMD Bass program, and reassembles full outputs.

Hardcoded problem shape: B=4, S=4096, D=128.
"""

import math
import numpy as np
from contextlib import ExitStack

import concourse.bass as bass
import concourse.bacc as bacc
import concourse.mybir as mybir
from concourse import tile
from concourse.bass_utils import run_bass_kernel_spmd

B, S, D = 4, 4096, 128
NCORES = 8
CORES_PER_B = NCORES // B          # 2
QSH = S // CORES_PER_B             # 2048 query rows per core
QT = 128                           # query tile rows
NQT = QSH // QT                    # 16 q tiles
NKC = S // 128                     # 32 k chunks of 128
SCORE_CHUNK = 1024                 # free-dim of one score psum chunk (2 banks)
NSC = S // SCORE_CHUNK             # 4 score chunks per q tile

FP = mybir.dt.float32
F32R = mybir.dt.float32r
BF = mybir.dt.bfloat16
U8 = mybir.dt.uint8

# precision knobs
SCORES_DT = F32R     # dtype for the q@k^T matmuls
PROJ_DT = FP         # dtype for the K/V projection matmuls
NM_DT = FP           # dtype of masked-exp tile nm (FP = accurate attn, BF = fast)

Exp = mybir.ActivationFunctionType.Exp
Alu = mybir.AluOpType


def _build_nc():
    nc = bacc.Bacc("TRN2", target_bir_lowering=False, debug=False)
    inp_b = nc.dram_tensor("inp_b", [S, D], FP, kind="ExternalInput").ap()
    w_k = nc.dram_tensor("w_k", [D, D], FP, kind="ExternalInput").ap()
    w_v = nc.dram_tensor("w_v", [D, D], FP, kind="ExternalInput").ap()
    mask_s = nc.dram_tensor("mask_s", [QSH, S], U8, kind="ExternalInput").ap()
    ident = nc.dram_tensor("ident", [128, 128], FP, kind="ExternalInput").ap()
    attn_s = nc.dram_tensor("attn_s", [QSH, S], FP, kind="ExternalOutput").ap()
    out_s = nc.dram_tensor("out_s", [QSH, D], FP, kind="ExternalOutput").ap()

    with tile.TileContext(nc) as tc:
        _kernel_body(tc, inp_b, w_k, w_v, mask_s, ident, attn_s, out_s)
    nc.compile()
    return nc


def _kernel_body(tc, inp_b, w_k, w_v, mask_s, ident_d, attn_s, out_s):
    nc = tc.nc
    with ExitStack() as ctx:
        const_pool = ctx.enter_context(tc.tile_pool(name="const", bufs=1))
        persist = ctx.enter_context(tc.tile_pool(name="persist", bufs=1))

        ident = const_pool.tile([128, 128], FP)
        nc.sync.dma_start(ident[:], ident_d[:])

        # persistent operands
        inpT = persist.tile([128, S], F32R)  # [d, s] = inp[b]^T (f32r-rounded)
        kT = persist.tile([128, S], F32R)    # [e, s] = K^T / sqrt(D)
        v_sb = persist.tile([128, S], BF)   # chunk-major V: cols j*128.. hold V[j*128+p, d]

        # ---- phase 0: load + transpose inp, project K^T and V ----
        with ExitStack() as p0:
            ld_pool = p0.enter_context(tc.tile_pool(name="p0_ld", bufs=3))
            ps_pool = p0.enter_context(
                tc.tile_pool(name="p0_ps", bufs=2, space="PSUM")
            )
            w_pool = p0.enter_context(tc.tile_pool(name="p0_w", bufs=1))

            # w_k^T, w_v^T
            wk_t = w_pool.tile([128, 128], FP)
            wv_t = w_pool.tile([128, 128], FP)
            wkT = w_pool.tile([128, 128], F32R)
            wvT = w_pool.tile([128, 128], F32R)
            nc.sync.dma_start(wk_t[:], w_k[:])
            nc.sync.dma_start(wv_t[:], w_v[:])
            wps = ps_pool.tile([128, 512], FP, tag="p0ps")
            nc.tensor.matmul(wps[:, 0:128], wk_t[:], ident[:], is_transpose=True,
                             start=True, stop=False)
            nc.tensor.matmul(wps[:, 128:256], wv_t[:], ident[:], is_transpose=True,
                             start=False, stop=True)
            nc.vector.tensor_copy(wkT[:], wps[:, 0:128])
            nc.vector.tensor_copy(wvT[:], wps[:, 128:256])

            # inp^T: 32 tile transposes, 4 per psum bank -> one copy per 512
            for g in range(NKC // 4):
                tps = ps_pool.tile([128, 512], FP, tag="p0ps")
                for t in range(4):
                    j = g * 4 + t
                    it = ld_pool.tile([128, 128], FP, tag="it")
                    nc.sync.dma_start(it[:], inp_b[j * 128:(j + 1) * 128, :])
                    nc.tensor.matmul(tps[:, t * 128:(t + 1) * 128], it[:], ident[:],
                                     is_transpose=True,
                                     start=(t == 0), stop=(t == 3))
                nc.vector.tensor_copy(inpT[:, g * 512:(g + 1) * 512], tps[:])

            # K^T = (w_k^T).T @ inp^T, scaled by 1/sqrt(D) on the psum->sbuf copy
            for jj in range(S // 512):
                kps = ps_pool.tile([128, 512], FP, tag="p0ps")
                nc.tensor.matmul(
                    kps[:],
                    wkT[:],
                    inpT[:, jj * 512:(jj + 1) * 512],
                    start=True, stop=True,
                )
                nc.scalar.mul(kT[:, jj * 512:(jj + 1) * 512], kps[:],
                              1.0 / math.sqrt(D))

            # V chunks (natural [s, d] layout, bf16), 4 matmuls per psum bank
            for g in range(NKC // 4):
                vps = ps_pool.tile([128, 512], FP, tag="p0ps")
                for t in range(4):
                    j = g * 4 + t
                    nc.tensor.matmul(
                        vps[:, t * 128:(t + 1) * 128],
                        inpT[:, j * 128:(j + 1) * 128],
                        wvT[:],
                        start=(t == 0), stop=(t == 3),
                    )
                nc.vector.tensor_copy(v_sb[:, g * 512:(g + 1) * 512], vps[:])

        # ---- main loop over q tiles ----
        m_pool = ctx.enter_context(tc.tile_pool(name="m", bufs=2))
        e_pool = ctx.enter_context(tc.tile_pool(name="e", bufs=2))
        nm_pool = ctx.enter_context(tc.tile_pool(name="nm", bufs=2))
        a_pool = ctx.enter_context(tc.tile_pool(name="attn", bufs=2))
        nmT_pool = ctx.enter_context(tc.tile_pool(name="nmT", bufs=2))
        st_pool = ctx.enter_context(tc.tile_pool(name="stats", bufs=4))
        o_pool = ctx.enter_context(tc.tile_pool(name="osb", bufs=2))

        s_psum = ctx.enter_context(tc.tile_pool(name="s_ps", bufs=2, space="PSUM"))
        t_psum = ctx.enter_context(tc.tile_pool(name="t_ps", bufs=2, space="PSUM"))
        o_psum = ctx.enter_context(tc.tile_pool(name="o_ps", bufs=2, space="PSUM"))

        for i in range(NQT):
            qlo = i * QT

            m_tile = m_pool.tile([128, S], U8, tag="m")
            nc.sync.dma_start(m_tile[:], mask_s[qlo:qlo + QT, :])

            # scores -> exp, in chunks of SCORE_CHUNK
            e_tile = e_pool.tile([128, S], FP, tag="e")
            for c in range(NSC):
                sps = s_psum.tile([128, SCORE_CHUNK], FP, tag="sps")
                for h in range(SCORE_CHUNK // 512):
                    col = c * SCORE_CHUNK + h * 512
                    nc.tensor.matmul(
                        sps[:, h * 512:(h + 1) * 512],
                        inpT[:, qlo:qlo + QT],
                        kT[:, col:col + 512],
                        start=True, stop=True,
                    )
                nc.scalar.activation(
                    e_tile[:, c * SCORE_CHUNK:(c + 1) * SCORE_CHUNK], sps[:], Exp
                )

            # nm = (m == 0) * e   (masked exp scores), ssum = sum_k nm
            nm_tile = nm_pool.tile([128, S], NM_DT, tag="nm")
            ssum = st_pool.tile([128, 1], FP, tag="ss")
            nc.vector.scalar_tensor_tensor(
                nm_tile[:], m_tile[:], 0.0, e_tile[:],
                op0=Alu.is_equal, op1=Alu.mult, accum_out=ssum[:],
            )
            rcp = st_pool.tile([128, 1], FP, tag="rc")
            nc.vector.reciprocal(rcp[:], ssum[:])

            # attn = nm / ssum  -> DRAM
            attn_tile = a_pool.tile([128, S], FP, tag="attn")
            nc.vector.tensor_scalar_mul(attn_tile[:], nm_tile[:], rcp[:])
            nc.sync.dma_start(attn_s[qlo:qlo + QT, :], attn_tile[:])

            # transpose nm into [k, q] chunks, cast to bf16 on the psum->sbuf
            # copy; one psum bank (2KB/partition) holds chunks_per_bank chunks
            nmT = nmT_pool.tile([128, S], BF, tag="nmT")
            cpb = 2048 // mybir.dt.size(NM_DT) // 128   # chunks per bank
            ident_t = ident[:].bitcast(NM_DT)
            for g in range(NKC // cpb):
                tps = t_psum.tile([128, cpb * 128], NM_DT, tag="tps")
                for t in range(cpb):
                    j = g * cpb + t
                    nc.tensor.matmul(
                        tps[:, t * 128:(t + 1) * 128],
                        nm_tile[:, j * 128:(j + 1) * 128],
                        ident_t,
                        is_transpose=True,
                        start=(t == 0), stop=(t == cpb - 1),
                    )
                cp = nmT[:, g * cpb * 128:(g + 1) * cpb * 128]
                if g % 2 == 0:
                    nc.vector.tensor_copy(cp, tps[:])
                else:
                    nc.scalar.copy(cp, tps[:])

            # outT[d, q] += sum_j V_j^T @ nmT_j   (bf16 matmuls)
            ops = o_psum.tile([128, 128], FP, tag="ops")
            for j in range(NKC):
                nc.tensor.matmul(
                    ops[:],
                    v_sb[:, j * 128:(j + 1) * 128],
                    nmT[:, j * 128:(j + 1) * 128],
                    start=(j == 0), stop=(j == NKC - 1),
                )

            # epilogue: outT -> SBUF -> transpose -> scale by 1/ssum -> DRAM
            oT_sb = o_pool.tile([128, 128], FP, tag="oT")
            nc.vector.tensor_copy(oT_sb[:], ops[:])
            ops2 = t_psum.tile([128, 512], FP, tag="tps")
            nc.tensor.matmul(
                ops2[:, 0:128], oT_sb[:], ident[:], is_transpose=True,
                start=True, stop=True,
            )
            out_sb = o_pool.tile([128, 128], FP, tag="out")
            nc.scalar.mul(out_sb[:], ops2[:, 0:128], rcp[:])
            nc.sync.dma_start(out_s[qlo:qlo + QT, :], out_sb[:])


_NC_CACHE = None


def _get_nc():
    global _NC_CACHE
    if _NC_CACHE is None:
        _NC_CACHE = _build_nc()
    return _NC_CACHE


def kernel(inp, w_k, w_v, mask, _trace=False, _trace_kwargs=None):
    inp = np.asarray(inp, dtype=np.float32)
    w_k = np.asarray(w_k, dtype=np.float32)
    w_v = np.asarray(w_v, dtype=np.float32)
    mask_u8 = np.ascontiguousarray(np.asarray(mask)).view(np.uint8)
    ident = np.eye(128, dtype=np.float32)

    nc = _get_nc()
    # One SPMD program always works on query rows 0..QSH of its inp_b. A
    # core handling the second query half (h=1) gets inp_b rolled by QSH
    # rows so those rows land at 0..QSH; the k axis (keys/values/mask
    # columns, attn columns) is then in rolled order for that core and the
    # attn columns are un-rolled during assembly below.
    in_maps = []
    for c in range(NCORES):
        b, h = c // CORES_PER_B, c % CORES_PER_B
        if h == 0:
            inp_c = np.ascontiguousarray(inp[b])
            mask_c = np.ascontiguousarray(mask_u8[b, :QSH, :])
        else:
            inp_c = np.ascontiguousarray(np.roll(inp[b], -QSH, axis=0))
            ms = mask_u8[b, QSH:, :]
            mask_c = np.ascontiguousarray(np.roll(ms, -QSH, axis=1))
        in_maps.append({
            "inp_b": inp_c,
            "w_k": w_k,
            "w_v": w_v,
            "mask_s": mask_c,
            "ident": ident,
        })

    res = run_bass_kernel_spmd(
        nc, in_maps, core_ids=list(range(NCORES)),
        trace=_trace, **(_trace_kwargs or {}),
    )

    out = np.empty((B, S, D), dtype=np.float32)
    attn = np.empty((B, S, S), dtype=np.float32)
    for c in range(NCORES):
        b, h = c // CORES_PER_B, c % CORES_PER_B
        a_c = res.results[c]["attn_s"]
        rows = slice(h * QSH, (h + 1) * QSH)
        if h == 0:
            attn[b, rows, :] = a_c
        else:
            attn[b, rows, QSH:] = a_c[:, :S - QSH]
            attn[b, rows, :QSH] = a_c[:, S - QSH:]
        out[b, rows, :] = res.results[c]["out_s"]

    if _trace:
        return (out, attn), res
    return (out, attn)


# revision 14
# speedup vs baseline: 1.2629x; 1.2629x over previous
"""ContextAttention TRN2 kernel: out, attn = softmax(mask(inp@Wk^T @ inp^T / sqrt(D))) ...

Full-input contract: kernel(**inputs) takes the complete tensors, shards
across 8 NeuronCores (2 cores per batch element, each handling 2048 query
rows), runs one SPMD Bass program, and reassembles full outputs.

Hardcoded problem shape: B=4, S=4096, D=128.
"""

import math
import numpy as np
from contextlib import ExitStack

import concourse.bass as bass
import concourse.bacc as bacc
import concourse.mybir as mybir
from concourse import tile
from concourse.bass_utils import run_bass_kernel_spmd

B, S, D = 4, 4096, 128
NCORES = 8
CORES_PER_B = NCORES // B          # 2
QSH = S // CORES_PER_B             # 2048 query rows per core
QT = 128                           # query tile rows
NQT = QSH // QT                    # 16 q tiles
NKC = S // 128                     # 32 k chunks of 128
SCORE_CHUNK = 1024                 # free-dim of one score psum chunk (2 banks)
NSC = S // SCORE_CHUNK             # 4 score chunks per q tile
GROUP = 2                          # q tiles batched per out-matmul group

FP = mybir.dt.float32
F32R = mybir.dt.float32r
BF = mybir.dt.bfloat16
U8 = mybir.dt.uint8

# precision knobs
SCORES_DT = F32R     # dtype for the q@k^T matmuls
PROJ_DT = FP         # dtype for the K/V projection matmuls
NM_DT = BF           # dtype of masked-exp tile nm (FP = accurate attn, BF = fast)

Exp = mybir.ActivationFunctionType.Exp
Alu = mybir.AluOpType


def _build_nc():
    nc = bacc.Bacc("TRN2", target_bir_lowering=False, debug=False)
    inp_b = nc.dram_tensor("inp_b", [S, D], FP, kind="ExternalInput").ap()
    w_k = nc.dram_tensor("w_k", [D, D], FP, kind="ExternalInput").ap()
    w_v = nc.dram_tensor("w_v", [D, D], FP, kind="ExternalInput").ap()
    mask_s = nc.dram_tensor("mask_s", [QSH, S], U8, kind="ExternalInput").ap()
    ident = nc.dram_tensor("ident", [128, 128], FP, kind="ExternalInput").ap()
    attn_s = nc.dram_tensor("attn_s", [QSH, S], FP, kind="ExternalOutput").ap()
    out_s = nc.dram_tensor("out_s", [QSH, D], FP, kind="ExternalOutput").ap()

    with tile.TileContext(nc) as tc:
        _kernel_body(tc, inp_b, w_k, w_v, mask_s, ident, attn_s, out_s)
    nc.compile()
    return nc


def _kernel_body(tc, inp_b, w_k, w_v, mask_s, ident_d, attn_s, out_s):
    nc = tc.nc
    with ExitStack() as ctx:
        const_pool = ctx.enter_context(tc.tile_pool(name="const", bufs=1))
        persist = ctx.enter_context(tc.tile_pool(name="persist", bufs=1))

        ident = const_pool.tile([128, 128], FP)
        nc.sync.dma_start(ident[:], ident_d[:])
        ident_nm = const_pool.tile([128, 128], NM_DT)
        nc.vector.tensor_copy(ident_nm[:], ident[:])

        # persistent operands
        inpT = persist.tile([128, S], F32R)  # [d, s] = inp[b]^T (f32r-rounded)
        kT = persist.tile([128, S], F32R)    # [e, s] = K^T / sqrt(D)
        v_sb = persist.tile([128, S], BF)   # chunk-major V: cols j*128.. hold V[j*128+p, d]

        # ---- phase 0: load + transpose inp, project K^T and V ----
        with ExitStack() as p0:
            ld_pool = p0.enter_context(tc.tile_pool(name="p0_ld", bufs=3))
            ps_pool = p0.enter_context(
                tc.tile_pool(name="p0_ps", bufs=2, space="PSUM")
            )
            w_pool = p0.enter_context(tc.tile_pool(name="p0_w", bufs=1))

            # w_k^T, w_v^T
            wk_t = w_pool.tile([128, 128], FP)
            wv_t = w_pool.tile([128, 128], FP)
            wkT = w_pool.tile([128, 128], F32R)
            wvT = w_pool.tile([128, 128], F32R)
            nc.sync.dma_start(wk_t[:], w_k[:])
            nc.sync.dma_start(wv_t[:], w_v[:])
            wps = ps_pool.tile([128, 512], FP, tag="p0ps")
            nc.tensor.matmul(wps[:, 0:128], wk_t[:], ident[:], is_transpose=True,
                             start=True, stop=False)
            nc.tensor.matmul(wps[:, 128:256], wv_t[:], ident[:], is_transpose=True,
                             start=False, stop=True)
            nc.vector.tensor_copy(wkT[:], wps[:, 0:128])
            nc.vector.tensor_copy(wvT[:], wps[:, 128:256])

            # inp^T: 32 tile transposes, 4 per psum bank -> one copy per 512
            for g in range(NKC // 4):
                tps = ps_pool.tile([128, 512], FP, tag="p0ps")
                for t in range(4):
                    j = g * 4 + t
                    it = ld_pool.tile([128, 128], FP, tag="it")
                    nc.sync.dma_start(it[:], inp_b[j * 128:(j + 1) * 128, :])
                    nc.tensor.matmul(tps[:, t * 128:(t + 1) * 128], it[:], ident[:],
                                     is_transpose=True,
                                     start=(t == 0), stop=(t == 3))
                nc.vector.tensor_copy(inpT[:, g * 512:(g + 1) * 512], tps[:])

            # K^T = (w_k^T).T @ inp^T, scaled by 1/sqrt(D) on the psum->sbuf copy
            for jj in range(S // 512):
                kps = ps_pool.tile([128, 512], FP, tag="p0ps")
                nc.tensor.matmul(
                    kps[:],
                    wkT[:],
                    inpT[:, jj * 512:(jj + 1) * 512],
                    start=True, stop=True,
                )
                nc.scalar.mul(kT[:, jj * 512:(jj + 1) * 512], kps[:],
                              1.0 / math.sqrt(D))

            # V chunks (natural [s, d] layout, bf16), 4 matmuls per psum bank
            for g in range(NKC // 4):
                vps = ps_pool.tile([128, 512], FP, tag="p0ps")
                for t in range(4):
                    j = g * 4 + t
                    nc.tensor.matmul(
                        vps[:, t * 128:(t + 1) * 128],
                        inpT[:, j * 128:(j + 1) * 128],
                        wvT[:],
                        start=(t == 0), stop=(t == 3),
                    )
                nc.vector.tensor_copy(v_sb[:, g * 512:(g + 1) * 512], vps[:])

        # ---- main loop over q tiles ----
        m_pool = ctx.enter_context(tc.tile_pool(name="m", bufs=2))
        e_pool = ctx.enter_context(tc.tile_pool(name="e", bufs=2))
        nm_pool = ctx.enter_context(tc.tile_pool(name="nm", bufs=2))
        a_pool = ctx.enter_context(tc.tile_pool(name="attn", bufs=2))
        nmT_pool = ctx.enter_context(tc.tile_pool(name="nmT", bufs=2))
        st_pool = ctx.enter_context(tc.tile_pool(name="stats", bufs=4))
        o_pool = ctx.enter_context(tc.tile_pool(name="osb", bufs=2))

        s_psum = ctx.enter_context(tc.tile_pool(name="s_ps", bufs=2, space="PSUM"))
        t_psum = ctx.enter_context(tc.tile_pool(name="t_ps", bufs=2, space="PSUM"))
        o_psum = ctx.enter_context(tc.tile_pool(name="o_ps", bufs=2, space="PSUM"))

        for i in range(NQT):
            qlo = i * QT

            m_tile = m_pool.tile([128, S], U8, tag="m")
            nc.sync.dma_start(m_tile[:], mask_s[qlo:qlo + QT, :])

            # scores -> exp, in chunks of SCORE_CHUNK
            e_tile = e_pool.tile([128, S], FP, tag="e")
            for c in range(NSC):
                sps = s_psum.tile([128, SCORE_CHUNK], FP, tag="sps")
                for h in range(SCORE_CHUNK // 512):
                    col = c * SCORE_CHUNK + h * 512
                    nc.tensor.matmul(
                        sps[:, h * 512:(h + 1) * 512],
                        inpT[:, qlo:qlo + QT],
                        kT[:, col:col + 512],
                        start=True, stop=True,
                    )
                nc.scalar.activation(
                    e_tile[:, c * SCORE_CHUNK:(c + 1) * SCORE_CHUNK], sps[:], Exp
                )

            # nm = (m == 0) * e   (masked exp scores), ssum = sum_k nm
            nm_tile = nm_pool.tile([128, S], NM_DT, tag="nm")
            ssum = st_pool.tile([128, 1], FP, tag="ss")
            nc.vector.scalar_tensor_tensor(
                nm_tile[:], m_tile[:], 0.0, e_tile[:],
                op0=Alu.is_equal, op1=Alu.mult, accum_out=ssum[:],
            )
            rcp = st_pool.tile([128, 1], FP, tag="rc")
            nc.vector.reciprocal(rcp[:], ssum[:])

            # attn = nm / ssum  -> DRAM
            attn_tile = a_pool.tile([128, S], FP, tag="attn")
            nc.vector.tensor_scalar_mul(attn_tile[:], nm_tile[:], rcp[:])
            nc.sync.dma_start(attn_s[qlo:qlo + QT, :], attn_tile[:])

            # transpose nm into [k, q] chunks (bf16); one psum bank holds 8
            # chunks. Chunks land group-interleaved in nmT_g so the out
            # matmuls can consume [128, GROUP*128]-wide rhs slices.
            ti = i % GROUP
            if ti == 0:
                nmT_g = nmT_pool.tile([128, GROUP * S], BF, tag="nmT")
                grp_rcp = []
            grp_rcp.append(rcp)
            cpb = 8
            for g in range(NKC // cpb):
                tps = t_psum.tile([128, cpb * 128], NM_DT, tag="tps")
                for t in range(cpb):
                    j = g * cpb + t
                    nc.tensor.matmul(
                        tps[:, t * 128:(t + 1) * 128],
                        nm_tile[:, j * 128:(j + 1) * 128],
                        ident_nm[:],
                        is_transpose=True,
                        start=(t == 0), stop=(t == cpb - 1),
                    )
                # dest: for j in [g*8,(g+1)*8): cols j*GROUP*128 + ti*128 + q
                dst = nmT_g.rearrange(
                    "p (j t q) -> p j t q", j=NKC, t=GROUP
                )[:, g * cpb:(g + 1) * cpb, ti, :]
                if g % 2 == 0:
                    nc.vector.tensor_copy(dst, tps[:])
                else:
                    nc.scalar.copy(dst, tps[:])

            if ti == GROUP - 1:
                # outT[d, (t,q)] += sum_j V_j^T @ nmT_j  (bf16, N=GROUP*128)
                ops = o_psum.tile([128, GROUP * 128], FP, tag="ops")
                for j in range(NKC):
                    nc.tensor.matmul(
                        ops[:],
                        v_sb[:, j * 128:(j + 1) * 128],
                        nmT_g[:, j * GROUP * 128:(j + 1) * GROUP * 128],
                        start=(j == 0), stop=(j == NKC - 1),
                    )
                # epilogue: outT -> SBUF -> per-tile transpose+scale -> DRAM
                oT_sb = o_pool.tile([128, GROUP * 128], FP, tag="oT")
                nc.vector.tensor_copy(oT_sb[:], ops[:])
                ops2 = t_psum.tile([128, 512], FP, tag="tps")
                for t in range(GROUP):
                    nc.tensor.matmul(
                        ops2[:, t * 128:(t + 1) * 128],
                        oT_sb[:, t * 128:(t + 1) * 128], ident[:],
                        is_transpose=True,
                        start=(t == 0), stop=(t == GROUP - 1),
                    )
                out_sb = o_pool.tile([128, GROUP * 128], FP, tag="out")
                for t in range(GROUP):
                    nc.scalar.mul(out_sb[:, t * 128:(t + 1) * 128],
                                  ops2[:, t * 128:(t + 1) * 128], grp_rcp[t][:])
                for t in range(GROUP):
                    it0 = i - GROUP + 1 + t
                    nc.sync.dma_start(
                        out_s[it0 * QT:(it0 + 1) * QT, :],
                        out_sb[:, t * 128:(t + 1) * 128],
                    )


_NC_CACHE = None


def _get_nc():
    global _NC_CACHE
    if _NC_CACHE is None:
        _NC_CACHE = _build_nc()
    return _NC_CACHE


def kernel(inp, w_k, w_v, mask, _trace=False, _trace_kwargs=None):
    inp = np.asarray(inp, dtype=np.float32)
    w_k = np.asarray(w_k, dtype=np.float32)
    w_v = np.asarray(w_v, dtype=np.float32)
    mask_u8 = np.ascontiguousarray(np.asarray(mask)).view(np.uint8)
    ident = np.eye(128, dtype=np.float32)

    nc = _get_nc()
    # One SPMD program always works on query rows 0..QSH of its inp_b. A
    # core handling the second query half (h=1) gets inp_b rolled by QSH
    # rows so those rows land at 0..QSH; the k axis (keys/values/mask
    # columns, attn columns) is then in rolled order for that core and the
    # attn columns are un-rolled during assembly below.
    in_maps = []
    for c in range(NCORES):
        b, h = c // CORES_PER_B, c % CORES_PER_B
        if h == 0:
            inp_c = np.ascontiguousarray(inp[b])
            mask_c = np.ascontiguousarray(mask_u8[b, :QSH, :])
        else:
            inp_c = np.ascontiguousarray(np.roll(inp[b], -QSH, axis=0))
            ms = mask_u8[b, QSH:, :]
            mask_c = np.ascontiguousarray(np.roll(ms, -QSH, axis=1))
        in_maps.append({
            "inp_b": inp_c,
            "w_k": w_k,
            "w_v": w_v,
            "mask_s": mask_c,
            "ident": ident,
        })

    res = run_bass_kernel_spmd(
        nc, in_maps, core_ids=list(range(NCORES)),
        trace=_trace, **(_trace_kwargs or {}),
    )

    out = np.empty((B, S, D), dtype=np.float32)
    attn = np.empty((B, S, S), dtype=np.float32)
    for c in range(NCORES):
        b, h = c // CORES_PER_B, c % CORES_PER_B
        a_c = res.results[c]["attn_s"]
        rows = slice(h * QSH, (h + 1) * QSH)
        if h == 0:
            attn[b, rows, :] = a_c
        else:
            attn[b, rows, QSH:] = a_c[:, :S - QSH]
            attn[b, rows, :QSH] = a_c[:, S - QSH:]
        out[b, rows, :] = res.results[c]["out_s"]

    if _trace:
        return (out, attn), res
    return (out, attn)


# revision 16
# speedup vs baseline: 1.3365x; 1.0582x over previous
"""ContextAttention TRN2 kernel: out, attn = softmax(mask(inp@Wk^T @ inp^T / sqrt(D))) ...

Full-input contract: kernel(**inputs) takes the complete tensors, shards
across 8 NeuronCores (2 cores per batch element, each handling 2048 query
rows), runs one SPMD Bass program, and reassembles full outputs.

Hardcoded problem shape: B=4, S=4096, D=128.
"""

import math
import numpy as np
from contextlib import ExitStack

import concourse.bass as bass
import concourse.bacc as bacc
import concourse.mybir as mybir
from concourse import tile
from concourse.bass_utils import run_bass_kernel_spmd

B, S, D = 4, 4096, 128
NCORES = 8
CORES_PER_B = NCORES // B          # 2
QSH = S // CORES_PER_B             # 2048 query rows per core
QT = 128                           # query tile rows
NQT = QSH // QT                    # 16 q tiles
NKC = S // 128                     # 32 k chunks of 128
SCORE_CHUNK = 1024                 # free-dim of one score psum chunk (2 banks)
NSC = S // SCORE_CHUNK             # 4 score chunks per q tile
GROUP = 4                          # q tiles batched per out-matmul group

FP = mybir.dt.float32
F32R = mybir.dt.float32r
BF = mybir.dt.bfloat16
U8 = mybir.dt.uint8

# precision knobs
SCORES_DT = F32R     # dtype for the q@k^T matmuls
PROJ_DT = FP         # dtype for the K/V projection matmuls
NM_DT = BF           # dtype of masked-exp tile nm (FP = accurate attn, BF = fast)

Exp = mybir.ActivationFunctionType.Exp
Alu = mybir.AluOpType


def _build_nc():
    nc = bacc.Bacc("TRN2", target_bir_lowering=False, debug=False)
    inp_b = nc.dram_tensor("inp_b", [S, D], FP, kind="ExternalInput").ap()
    w_k = nc.dram_tensor("w_k", [D, D], FP, kind="ExternalInput").ap()
    w_v = nc.dram_tensor("w_v", [D, D], FP, kind="ExternalInput").ap()
    mask_s = nc.dram_tensor("mask_s", [QSH, S], U8, kind="ExternalInput").ap()
    ident = nc.dram_tensor("ident", [128, 128], FP, kind="ExternalInput").ap()
    pen = nc.dram_tensor("pen", [128, 128], mybir.dt.uint8, kind="ExternalInput").ap()
    attn_s = nc.dram_tensor("attn_s", [QSH, S], FP, kind="ExternalOutput").ap()
    out_s = nc.dram_tensor("out_s", [QSH, D], FP, kind="ExternalOutput").ap()

    with tile.TileContext(nc) as tc:
        _kernel_body(tc, inp_b, w_k, w_v, mask_s, ident, pen, attn_s, out_s)
    nc.compile()
    return nc


def _kernel_body(tc, inp_b, w_k, w_v, mask_s, ident_d, pen_d, attn_s, out_s):
    nc = tc.nc
    with ExitStack() as ctx:
        const_pool = ctx.enter_context(tc.tile_pool(name="const", bufs=1))
        persist = ctx.enter_context(tc.tile_pool(name="persist", bufs=1))

        ident = const_pool.tile([128, 128], FP)
        nc.sync.dma_start(ident[:], ident_d[:])
        ident_nm = const_pool.tile([128, 128], NM_DT)
        nc.vector.tensor_copy(ident_nm[:], ident[:])
        # -57344*I in e5m2 bytes; paired with the mask bytes read as e4m3
        # (0x01 -> 2^-9) one matmul adds -112 to each masked score.
        pen_t = const_pool.tile([128, 128], mybir.dt.uint8)
        nc.sync.dma_start(pen_t[:], pen_d[:])

        # persistent operands
        inpT = persist.tile([128, S], F32R)  # [d, s] = inp[b]^T (f32r-rounded)
        kT = persist.tile([128, S], F32R)    # [e, s] = K^T / sqrt(D)
        v_sb = persist.tile([128, S], BF)   # chunk-major V: cols j*128.. hold V[j*128+p, d]

        # ---- phase 0: load + transpose inp, project K^T and V ----
        with ExitStack() as p0:
            ld_pool = p0.enter_context(tc.tile_pool(name="p0_ld", bufs=3))
            ps_pool = p0.enter_context(
                tc.tile_pool(name="p0_ps", bufs=2, space="PSUM")
            )
            w_pool = p0.enter_context(tc.tile_pool(name="p0_w", bufs=1))

            # w_k^T, w_v^T
            wk_t = w_pool.tile([128, 128], FP)
            wv_t = w_pool.tile([128, 128], FP)
            wkT = w_pool.tile([128, 128], F32R)
            wvT = w_pool.tile([128, 128], F32R)
            nc.sync.dma_start(wk_t[:], w_k[:])
            nc.sync.dma_start(wv_t[:], w_v[:])
            wps = ps_pool.tile([128, 512], FP, tag="p0ps")
            nc.tensor.matmul(wps[:, 0:128], wk_t[:], ident[:], is_transpose=True,
                             start=True, stop=False)
            nc.tensor.matmul(wps[:, 128:256], wv_t[:], ident[:], is_transpose=True,
                             start=False, stop=True)
            nc.vector.tensor_copy(wkT[:], wps[:, 0:128])
            nc.vector.tensor_copy(wvT[:], wps[:, 128:256])

            # inp^T: 32 tile transposes, 4 per psum bank -> one copy per 512
            for g in range(NKC // 4):
                tps = ps_pool.tile([128, 512], FP, tag="p0ps")
                for t in range(4):
                    j = g * 4 + t
                    it = ld_pool.tile([128, 128], FP, tag="it")
                    nc.sync.dma_start(it[:], inp_b[j * 128:(j + 1) * 128, :])
                    nc.tensor.matmul(tps[:, t * 128:(t + 1) * 128], it[:], ident[:],
                                     is_transpose=True,
                                     start=(t == 0), stop=(t == 3))
                nc.vector.tensor_copy(inpT[:, g * 512:(g + 1) * 512], tps[:])

            # K^T = (w_k^T).T @ inp^T, scaled by 1/sqrt(D) on the psum->sbuf copy
            for jj in range(S // 512):
                kps = ps_pool.tile([128, 512], FP, tag="p0ps")
                nc.tensor.matmul(
                    kps[:],
                    wkT[:],
                    inpT[:, jj * 512:(jj + 1) * 512],
                    start=True, stop=True,
                )
                nc.scalar.mul(kT[:, jj * 512:(jj + 1) * 512], kps[:],
                              1.0 / math.sqrt(D))

            # V chunks (natural [s, d] layout, bf16), 4 matmuls per psum bank
            for g in range(NKC // 4):
                vps = ps_pool.tile([128, 512], FP, tag="p0ps")
                for t in range(4):
                    j = g * 4 + t
                    nc.tensor.matmul(
                        vps[:, t * 128:(t + 1) * 128],
                        inpT[:, j * 128:(j + 1) * 128],
                        wvT[:],
                        start=(t == 0), stop=(t == 3),
                    )
                nc.vector.tensor_copy(v_sb[:, g * 512:(g + 1) * 512], vps[:])

        # ---- main loop over q tiles ----
        m_pool = ctx.enter_context(tc.tile_pool(name="m", bufs=2))
        nm_pool = ctx.enter_context(tc.tile_pool(name="nm", bufs=2))
        a_pool = ctx.enter_context(tc.tile_pool(name="attn", bufs=2))
        nmT_pool = ctx.enter_context(tc.tile_pool(name="nmT", bufs=2))
        st_pool = ctx.enter_context(tc.tile_pool(name="stats", bufs=4))
        o_pool = ctx.enter_context(tc.tile_pool(name="osb", bufs=2))

        s_psum = ctx.enter_context(tc.tile_pool(name="s_ps", bufs=2, space="PSUM"))
        t_psum = ctx.enter_context(tc.tile_pool(name="t_ps", bufs=2, space="PSUM"))
        o_psum = ctx.enter_context(tc.tile_pool(name="o_ps", bufs=2, space="PSUM"))

        for i in range(NQT):
            qlo = i * QT

            m_tile = m_pool.tile([128, S], U8, tag="m")
            nc.sync.dma_start(m_tile[:], mask_s[qlo:qlo + QT, :])

            # scores (+fp8 mask penalty) -> exp with accumulated row sums.
            # nm = exp(score - 112*mask) is the masked softmax numerator.
            nm_tile = nm_pool.tile([128, S], NM_DT, tag="nm")
            ssum4 = st_pool.tile([128, NSC], FP, tag="ss4")
            m_e4 = m_tile[:].bitcast(mybir.dt.float8e4)
            pen_e5 = pen_t[:].bitcast(mybir.dt.float8e5)
            for c in range(NSC):
                sps = s_psum.tile([128, SCORE_CHUNK], FP, tag="sps")
                for h in range(SCORE_CHUNK // 512):
                    col = c * SCORE_CHUNK + h * 512
                    nc.tensor.matmul(
                        sps[:, h * 512:(h + 1) * 512],
                        inpT[:, qlo:qlo + QT],
                        kT[:, col:col + 512],
                        start=True, stop=False,
                    )
                    nc.tensor.matmul(
                        sps[:, h * 512:(h + 1) * 512],
                        pen_e5,
                        m_e4[:, col:col + 512],
                        start=False, stop=True,
                    )
                nc.scalar.activation(
                    nm_tile[:, c * SCORE_CHUNK:(c + 1) * SCORE_CHUNK], sps[:], Exp,
                    accum_out=ssum4[:, c:c + 1],
                )
            ssum = st_pool.tile([128, 1], FP, tag="ss")
            nc.vector.tensor_reduce(ssum[:], ssum4[:], axis=mybir.AxisListType.X,
                                    op=Alu.add)
            rcp = st_pool.tile([128, 1], FP, tag="rc")
            nc.vector.reciprocal(rcp[:], ssum[:])

            # attn = nm / ssum  -> DRAM
            attn_tile = a_pool.tile([128, S], FP, tag="attn")
            nc.vector.tensor_scalar_mul(attn_tile[:], nm_tile[:], rcp[:])
            nc.sync.dma_start(attn_s[qlo:qlo + QT, :], attn_tile[:])

            # transpose nm into [k, q] chunks (bf16); one psum bank holds 8
            # chunks. Chunks land group-interleaved in nmT_g so the out
            # matmuls can consume [128, GROUP*128]-wide rhs slices.
            ti = i % GROUP
            if ti == 0:
                nmT_g = nmT_pool.tile([128, GROUP * S], BF, tag="nmT")
                grp_rcp = []
            grp_rcp.append(rcp)
            cpb = 8
            for g in range(NKC // cpb):
                tps = t_psum.tile([128, cpb * 128], NM_DT, tag="tps")
                for t in range(cpb):
                    j = g * cpb + t
                    nc.tensor.matmul(
                        tps[:, t * 128:(t + 1) * 128],
                        nm_tile[:, j * 128:(j + 1) * 128],
                        ident_nm[:],
                        is_transpose=True,
                        start=(t == 0), stop=(t == cpb - 1),
                    )
                # dest: for j in [g*8,(g+1)*8): cols j*GROUP*128 + ti*128 + q
                dst = nmT_g.rearrange(
                    "p (j t q) -> p j t q", j=NKC, t=GROUP
                )[:, g * cpb:(g + 1) * cpb, ti, :]
                if g % 2 == 0:
                    nc.vector.tensor_copy(dst, tps[:])
                else:
                    nc.scalar.copy(dst, tps[:])

            if ti == GROUP - 1:
                # outT[d, (t,q)] += sum_j V_j^T @ nmT_j  (bf16, N=GROUP*128)
                ops = o_psum.tile([128, GROUP * 128], FP, tag="ops")
                for j in range(NKC):
                    nc.tensor.matmul(
                        ops[:],
                        v_sb[:, j * 128:(j + 1) * 128],
                        nmT_g[:, j * GROUP * 128:(j + 1) * GROUP * 128],
                        start=(j == 0), stop=(j == NKC - 1),
                    )
                # epilogue: outT -> SBUF -> per-tile transpose+scale -> DRAM
                oT_sb = o_pool.tile([128, GROUP * 128], FP, tag="oT")
                nc.vector.tensor_copy(oT_sb[:], ops[:])
                ops2 = t_psum.tile([128, 512], FP, tag="tps")
                for t in range(GROUP):
                    nc.tensor.matmul(
                        ops2[:, t * 128:(t + 1) * 128],
                        oT_sb[:, t * 128:(t + 1) * 128], ident[:],
                        is_transpose=True,
                        start=(t == 0), stop=(t == GROUP - 1),
                    )
                out_sb = o_pool.tile([128, GROUP * 128], FP, tag="out")
                for t in range(GROUP):
                    nc.scalar.mul(out_sb[:, t * 128:(t + 1) * 128],
                                  ops2[:, t * 128:(t + 1) * 128], grp_rcp[t][:])
                for t in range(GROUP):
                    it0 = i - GROUP + 1 + t
                    nc.sync.dma_start(
                        out_s[it0 * QT:(it0 + 1) * QT, :],
                        out_sb[:, t * 128:(t + 1) * 128],
                    )


_NC_CACHE = None


def _get_nc():
    global _NC_CACHE
    if _NC_CACHE is None:
        _NC_CACHE = _build_nc()
    return _NC_CACHE


def kernel(inp, w_k, w_v, mask, _trace=False, _trace_kwargs=None):
    inp = np.asarray(inp, dtype=np.float32)
    w_k = np.asarray(w_k, dtype=np.float32)
    w_v = np.asarray(w_v, dtype=np.float32)
    mask_u8 = np.ascontiguousarray(np.asarray(mask)).view(np.uint8)
    ident = np.eye(128, dtype=np.float32)
    # -57344 in e5m2 is byte 0xFB; off-diagonal zeros
    pen = (np.eye(128) * 0xFB).astype(np.uint8)

    nc = _get_nc()
    # One SPMD program always works on query rows 0..QSH of its inp_b. A
    # core handling the second query half (h=1) gets inp_b rolled by QSH
    # rows so those rows land at 0..QSH; the k axis (keys/values/mask
    # columns, attn columns) is then in rolled order for that core and the
    # attn columns are un-rolled during assembly below.
    in_maps = []
    for c in range(NCORES):
        b, h = c // CORES_PER_B, c % CORES_PER_B
        if h == 0:
            inp_c = np.ascontiguousarray(inp[b])
            mask_c = np.ascontiguousarray(mask_u8[b, :QSH, :])
        else:
            inp_c = np.ascontiguousarray(np.roll(inp[b], -QSH, axis=0))
            ms = mask_u8[b, QSH:, :]
            mask_c = np.ascontiguousarray(np.roll(ms, -QSH, axis=1))
        in_maps.append({
            "inp_b": inp_c,
            "w_k": w_k,
            "w_v": w_v,
            "mask_s": mask_c,
            "ident": ident,
            "pen": pen,
        })

    res = run_bass_kernel_spmd(
        nc, in_maps, core_ids=list(range(NCORES)),
        trace=_trace, **(_trace_kwargs or {}),
    )

    out = np.empty((B, S, D), dtype=np.float32)
    attn = np.empty((B, S, S), dtype=np.float32)
    for c in range(NCORES):
        b, h = c // CORES_PER_B, c % CORES_PER_B
        a_c = res.results[c]["attn_s"]
        rows = slice(h * QSH, (h + 1) * QSH)
        if h == 0:
            attn[b, rows, :] = a_c
        else:
            attn[b, rows, QSH:] = a_c[:, :S - QSH]
            attn[b, rows, :QSH] = a_c[:, S - QSH:]
        out[b, rows, :] = res.results[c]["out_s"]

    if _trace:
        return (out, attn), res
    return (out, attn)
